# revision 1
# baseline (speedup 1.0000x reference)
"""Trainium2 Bass kernel for the dense branch-MLP problem.

Computes: out[b,o] = sum_n relu((s[b,:] - v[n,:]) @ W[n].T + bias[n])[o]
with B=1024, N=64, D=512, OUT=2048 in fp32.

Sharding: expert-style across the N=64 branch axis -> 8 branches per core.
Each core computes a full [B, OUT] partial sum over its 8 branches; the
host sums the 8 partials (the unshard step).

Per-core kernel (PE-bound, ~17.2 GFLOP at fp32r rates):
  - s^T resident in SBUF as 4 d-chunks [128, 1024]
  - per branch: offs = s^T - v_n (VectorE tensor_scalar, per-partition
    scalar), stream W[n]^T tiles as matmul stationary operands, accumulate
    over the 4 d-chunks in PSUM (8 interleaved bank groups so PE starts as
    soon as the first weight/offset chunks land), relu+bias on ScalarE,
    branch-sum on VectorE, per-(o,b)-tile output DMA.
  - matmuls run in float32r (fp22 internal) at 1 cycle/row since the
    moving free dim is 512 -> full bf16-class PE throughput with ~1e-4
    relative accuracy vs the fp32 reference.
  - a tiny-matmul warmup burst during the startup DMA window brings the
    PE HAM clock gate to 8/8 (2.4 GHz) before the first real matmul.

Cost-model timeline: ~235.6 us/core (PE busy ~221 us = 94%, vs a 218.5 us
theoretical floor for 1024 N=512 matmuls); validated on hardware
early-session at ~+3% (269.7 us measured vs 261.4 us predicted for the
baseline version of this kernel). Critical ordering detail: the bias DMA
loads FIRST — it gates the batch-0 relu drain and, through PSUM slot
recycling, every later matmul batch.
"""

import numpy as np

import concourse.bacc as bacc
import concourse.mybir as mybir
import concourse.tile as tile
from concourse.bass_utils import run_bass_kernel_spmd

B, N, D, OUT = 1024, 64, 512, 2048
N_CORES = 8
NL = N // N_CORES  # branches per core
DC = D // 128  # d chunks (4)
OT = OUT // 128  # o tiles (16)
BT = B // 512  # b free-dim tiles (2)

F32 = mybir.dt.float32
F32R = mybir.dt.float32r
BF16 = mybir.dt.bfloat16
RELU = mybir.ActivationFunctionType.Relu
IDENT = mybir.ActivationFunctionType.Identity

_cache = {}


def build(repeat: int = 1):
    """Build + compile the per-core Bass program. Cached per `repeat`."""
    if repeat in _cache:
        return _cache[repeat]

    nc = bacc.Bacc(
        "TRN2",
        target_bir_lowering=False,
        debug=False,
        num_devices=N_CORES,
    )

    wt_d = nc.dram_tensor("wt", [NL, 128, DC * OUT], F32R, kind="ExternalInput").ap()
    st_d = nc.dram_tensor("st", [128, DC * B], F32, kind="ExternalInput").ap()
    negv_d = nc.dram_tensor("negv", [128, NL * DC], F32, kind="ExternalInput").ap()
    bias_d = nc.dram_tensor("bias", [128, NL * OT], F32, kind="ExternalInput").ap()
    out_d = nc.dram_tensor("out", [OUT, B], F32, kind="ExternalOutput").ap()

    # o-range chunks per weight DMA: each chunk delivers o_tiles for all DC
    # d-chunks so matmul groups become ready progressively.
    WCH = 8  # wt DMA chunks per branch
    OT_PER_CH = OT // WCH

    with tile.TileContext(nc) as tc:
        with (
            tc.tile_pool(name="const", bufs=1) as const_pool,
            tc.tile_pool(name="acc", bufs=1) as acc_pool,
            tc.tile_pool(name="offs", bufs=2) as offs_pool,
            tc.tile_pool(name="wt", bufs=2) as wt_pool,
            tc.tile_pool(name="tmp", bufs=6) as tmp_pool,
            tc.tile_pool(name="psum", bufs=8, space="PSUM") as psum_pool,
        ):
            def wt_chunk_dma(wt, n, j, nch=WCH):
                wt3 = wt[:].rearrange("p (c o) -> p c o", c=DC)
                wd3 = wt_d[n].rearrange("p (c o) -> p c o", c=DC)
                osz = (OT // nch) * 128
                nc.sync.dma_start(
                    wt3[:, :, j * osz : (j + 1) * osz],
                    wd3[:, :, j * osz : (j + 1) * osz],
                )

            # Startup order matters: the first batch's c-outer matmuls need
            # ALL FOUR offs chunks (hence all of st) within ~7us of the first
            # matmul, while weight chunks are consumed at only ~1.7us each.
            # So: st0 + the first weight chunk to start PE, then the REST of
            # st immediately (offsets pace the first batch), then the
            # remaining branch-0 weight chunks.
            negv = const_pool.tile([128, NL * DC], F32, name="negv")
            nc.sync.dma_start(negv[:], negv_d[:])
            # bias is tiny but gates the batch-0 relu drain (and through PSUM
            # slot recycling, every later batch) -> load it FIRST.
            bias = const_pool.tile([128, NL * OT], F32, name="bias")
            nc.sync.dma_start(bias[:], bias_d[:])
            st = const_pool.tile([128, DC * B], F32, name="st")
            wt0 = wt_pool.tile([128, DC * OUT], F32R, name="wt_t", tag="wt_t")
            nc.sync.dma_start(st[:, 0:B], st_d[:, 0:B])
            wt_chunk_dma(wt0, 0, 0)
            wt_chunk_dma(wt0, 0, 1)
            for c in range(1, DC):
                nc.sync.dma_start(
                    st[:, c * B : (c + 1) * B], st_d[:, c * B : (c + 1) * B]
                )
            for j in range(2, WCH):
                wt_chunk_dma(wt0, 0, j)

            acc = [
                acc_pool.tile([128, B], F32, name=f"acc{ot}", tag=f"acc{ot}")
                for ot in range(OT)
            ]

            # PE warmup: a burst of tiny matmuls on scratch data during the
            # startup DMA window, so the HAM clock gate reaches 8/8 (2.4 GHz)
            # before the first real matmul issues.
            scr = const_pool.tile([128, 128], BF16, name="scr")
            nc.vector.memset(scr[:], 0.0)
            wps = psum_pool.tile([128, 512], F32, name="wps", tag="ps")
            for _ in range(56):
                nc.tensor.matmul(
                    wps[0:64, 0:64], scr[:, 0:64], scr[:, 64:128], start=True, stop=True
                )

            def load_wt(n):
                wt = wt_pool.tile([128, DC * OUT], F32R, name="wt_t", tag="wt_t")
                for j in range(WCH):
                    wt_chunk_dma(wt, n, j)
                return wt

            def make_offs(n, dt=F32R):
                offs = offs_pool.tile([128, DC * B], dt, name="offs", tag="offs")
                for c in range(DC):
                    nc.vector.tensor_scalar_add(
                        offs[:, c * B : (c + 1) * B],
                        st[:, c * B : (c + 1) * B],
                        negv[:, n * DC + c : n * DC + c + 1],
                    )
                return offs

            groups = [(ot, bt) for ot in range(OT) for bt in range(BT)]
            BATCH = 8  # interleaved psum groups (= psum banks)

            def drain_group(n, ps, ot, bt):
                b_ap = bias[:, n * OT + ot : n * OT + ot + 1]
                if n == 0:
                    nc.scalar.activation(
                        acc[ot][:, bt * 512 : bt * 512 + 512],
                        ps[:],
                        RELU,
                        bias=b_ap,
                        scale=1.0,
                    )
                else:
                    tmp = tmp_pool.tile([128, 512], F32, name="tmp", tag="tmp")
                    nc.scalar.activation(tmp[:], ps[:], RELU, bias=b_ap, scale=1.0)
                    nc.vector.tensor_add(
                        acc[ot][:, bt * 512 : bt * 512 + 512],
                        acc[ot][:, bt * 512 : bt * 512 + 512],
                        tmp[:],
                    )
                if n == NL - 1:
                    nc.sync.dma_start(
                        out_d[ot * 128 : (ot + 1) * 128, bt * 512 : bt * 512 + 512],
                        acc[ot][:, bt * 512 : bt * 512 + 512],
                    )

            def body(iv=None):
                for n in range(NL):
                    wt = wt0 if n == 0 else load_wt(n)
                    offs = make_offs(n)

                    last_branch = n == NL - 1
                    for g0 in range(0, len(groups), BATCH):
                        batch = groups[g0 : g0 + BATCH]
                        last_batch = last_branch
                        pss = [
                            psum_pool.tile([128, 512], F32, name="ps", tag="ps")
                            for _ in batch
                        ]
                        if last_batch:
                            # c-inner: groups finish one at a time so the
                            # ACT/DVE/DMA drain trickles instead of bunching
                            # after the final matmul.
                            for ps, (ot, bt) in zip(pss, batch):
                                for c in range(DC):
                                    nc.tensor.matmul(
                                        ps[:],
                                        wt[:, c * OUT + ot * 128 : c * OUT + (ot + 1) * 128],
                                        offs[:, c * B + bt * 512 : c * B + bt * 512 + 512],
                                        start=(c == 0),
                                        stop=(c == DC - 1),
                                    )
                                drain_group(n, ps, ot, bt)
                        else:
                            # d-chunk outer, group inner: PE starts as soon as
                            # the first offs/wt chunks land; later chunks
                            # stream in behind.
                            for c in range(DC):
                                for ps, (ot, bt) in zip(pss, batch):
                                    nc.tensor.matmul(
                                        ps[:],
                                        wt[:, c * OUT + ot * 128 : c * OUT + (ot + 1) * 128],
                                        offs[:, c * B + bt * 512 : c * B + bt * 512 + 512],
                                        start=(c == 0),
                                        stop=(c == DC - 1),
                                    )
                            for ps, (ot, bt) in zip(pss, batch):
                                drain_group(n, ps, ot, bt)

            if repeat == 1:
                body()
            else:
                with tc.For_i(0, repeat, 1):
                    body()

    nc.compile()
    _cache[repeat] = nc
    return nc


def prep_inputs(semantic_vec, vertices, W, b):
    """Host-side layout transforms -> per-core input maps."""
    semantic_vec = np.asarray(semantic_vec, dtype=np.float32)
    vertices = np.asarray(vertices, dtype=np.float32)
    W = np.asarray(W, dtype=np.float32)
    b = np.asarray(b, dtype=np.float32)

    # st[p, c*B + bb] = s[bb, c*128+p]
    st = np.ascontiguousarray(
        semantic_vec.reshape(B, DC, 128).transpose(2, 1, 0).reshape(128, DC * B)
    )
    # wt[n, p, c*OUT + o] = W[n, o, c*128+p]
    wt = np.ascontiguousarray(
        W.reshape(N, OUT, DC, 128).transpose(0, 3, 2, 1).reshape(N, 128, DC * OUT)
    )
    # negv[p, nl*DC + c] = -v[n0+nl, c*128+p]
    negv = np.ascontiguousarray(
        (-vertices).reshape(N_CORES, NL, DC, 128).transpose(0, 3, 1, 2).reshape(N_CORES, 128, NL * DC)
    )
    # bias[p, nl*OT + ot] = b[n0+nl, ot*128+p]
    bias = np.ascontiguousarray(
        b.reshape(N_CORES, NL, OT, 128).transpose(0, 3, 1, 2).reshape(N_CORES, 128, NL * OT)
    )

    in_maps = []
    for core in range(N_CORES):
        in_maps.append(
            {
                "wt": wt[core * NL : (core + 1) * NL],
                "st": st,
                "negv": negv[core],
                "bias": bias[core],
            }
        )
    return in_maps


def kernel(semantic_vec, vertices, W, b):
    nc = build(repeat=1)
    in_maps = prep_inputs(semantic_vec, vertices, W, b)
    res = run_bass_kernel_spmd(nc, in_maps, core_ids=list(range(N_CORES)))
    total = np.zeros((OUT, B), dtype=np.float32)
    for core in range(N_CORES):
        total += res.results[core]["out"]
    return np.ascontiguousarray(total.T)



# revision 28
# speedup vs baseline: 1.9885x; 1.9885x over previous
"""Trainium2 Bass kernel for the dense branch-MLP problem (fp8 DoubleRow).

Computes: out[b,o] = sum_n relu((s[b,:] - v[n,:]) @ W[n].T + bias[n])[o]
with B=1024, N=64, D=512, OUT=2048 in fp32, graded at rel_absmax < 2e-2.

Sharding: expert-style across the N=64 branch axis -> 8 branches per core.
Each core computes a full [B, OUT] partial sum over its 8 branches; the
host sums the 8 partials and descales (the unshard step).

Math restructure (host side):
  (s - v_n) @ W_n^T + b_n  ==  s @ W_n^T + c_n,   c_n = b_n - v_n @ W_n^T
so the device never materializes offsets; c_n folds into the relu bias.
Both s and (aw*W) are quantized to fp8 e4m3 on the host (aw=16 keeps W out
of the subnormal range); full-chain emulated rel_absmax = 1.35e-2.

Per-core device schedule (o-tile OUTER, branch inner, so each o-tile's
output DMA fires early and the serial DMA pipeline never backlogs at the
end):
  - per (ot, nl): 4 fp8 DoubleRow matmuls (0.5 cyc/row, 2 k-tiles each)
    accumulate psum [128,1024] = the branch response for one o-tile,
  - relu+bias psum -> bf16 on ACT (activation) or DVE (tensor_scalar
    (x+c) max 0), statically split,
  - the 7 branch-sum adds per o-tile run on three tracks: DVE
    tensor_tensor (bf16 2x mode), gpsimd accumulating DMAs, and gpsimd
    tensor_tensor; out DMA per o-tile right after its last add.
Weights stream as (branch, o-quad) chunks relaid out on the host so each
chunk is a contiguous 2KB-per-partition DMA.
"""

import numpy as np
import ml_dtypes

import concourse.bacc as bacc
import concourse.mybir as mybir
import concourse.tile as tile
from concourse.bass_utils import run_bass_kernel_spmd

B, N, D, OUT = 1024, 64, 512, 2048
N_CORES = 8
NL = N // N_CORES  # branches per core (8)
DC = D // 128      # contraction k-tiles (4)
OT = OUT // 128    # o tiles (16)
NQ = 4             # weight-stream quads (OT/4)
AW = 16.0          # host-side weight scale before fp8 quantization

F32 = mybir.dt.float32
BF16 = mybir.dt.bfloat16
F8 = mybir.dt.float8e4
RELU = mybir.ActivationFunctionType.Relu
DR = mybir.MatmulPerfMode.DoubleRow
ADD = mybir.AluOpType.add
MAX = mybir.AluOpType.max


def _spread(total, picks):
    """Boolean mask of length `total` with `picks` Trues spread evenly."""
    return [(i * picks) // total != ((i - 1) * picks) // total for i in range(total)]


# -- static schedules (tuned against the cost-model sim) --------------------
# relu engine per unit (unit = ot*8+nl): 'A' = ACT activation, 'D' = DVE.
N_DVE_RELU = 45
_RELU_SCHED = ["D" if d else "A" for d in _spread(128, N_DVE_RELU)]

# Branch-sum per ot is a depth-3 tree over {acc(=relu nl0), t1..t7}:
#   L1: t1+=t2   t3+=t4   t5+=t6   acc+=t7     (emitted as relus complete)
#   L2: t1+=t3   acc+=t5
#   L3: acc+=t1  -> out DMA
# Each slot's engine per ot: 'V' = DVE tensor_tensor, 'M' = gpsimd
# accumulating DMA (high latency, leaves only), 'P' = gpsimd TT.
def _slot_pat(ot, slot):
    if ot >= OT - 2:
        # keep the serial DMA pipeline clear for the final out DMAs
        return "P" if slot in (0, 2) else "V"
    if ot == OT - 3 and slot >= 2:
        return "P" if slot == 2 else "V"
    if slot in (0, 1):           # L1a, L1b (leaves: latency-tolerant)
        return "M"
    if slot == 2:                # L1c
        return "P" if ot % 3 == 1 else "M"
    if slot == 3:                # L1d (acc chain start, emitted at ot end)
        return "V"
    if slot == 4:                # L2a (lagged one ot)
        return "M" if ot % 2 else "V"
    if slot == 5:                # L2b (lagged one ot)
        return "V" if ot % 2 else "M"
    return "V"                   # L3 (feeds the out DMA)

_cache = {}


def build(repeat: int = 1):
    """Build + compile the per-core Bass program. Cached per `repeat`."""
    if repeat in _cache:
        return _cache[repeat]

    nc = bacc.Bacc(
        "TRN2",
        target_bir_lowering=False,
        debug=False,
        num_devices=N_CORES,
    )

    # weights pre-chunked on host: [nl, quad, 128, DC*512] fp8
    wt_d = nc.dram_tensor("wt", [NL, NQ, 128, DC * 512], F8, kind="ExternalInput").ap()
    st_d = nc.dram_tensor("st", [128, DC * B], F8, kind="ExternalInput").ap()
    cb_d = nc.dram_tensor("cb", [128, NL * OT], F32, kind="ExternalInput").ap()
    out_d = nc.dram_tensor("out", [128, OT * B], BF16, kind="ExternalOutput").ap()

    with tile.TileContext(nc) as tc:
        with (
            tc.tile_pool(name="const", bufs=1) as const_pool,
            tc.tile_pool(name="acc", bufs=4) as acc_pool,
            tc.tile_pool(name="tmp", bufs=3) as tmp_pool,
            tc.tile_pool(name="wt", bufs=1) as wt_pool,
            tc.tile_pool(name="psum", bufs=4, space="PSUM") as psum_pool,
        ):
            # Startup order: cb gates the first relu drain; s8 + the first
            # weight chunks gate the first matmuls.
            cb = const_pool.tile([128, NL * OT], F32, name="cb")
            nc.sync.dma_start(cb[:], cb_d[:])
            st = const_pool.tile([128, DC, B], F8, name="st")
            st_d3 = st_d.rearrange("p (c b) -> p c b", c=DC)
            for c in range(DC):
                nc.sync.dma_start(st[:, c, :], st_d3[:, c, :])

            # one resident weight tile per branch, filled quad-by-quad
            wts = [
                wt_pool.tile([128, DC, OUT], F8, name=f"wt{nl}", tag=f"wt{nl}")
                for nl in range(NL)
            ]

            def wt_chunk_dma(nl, q):
                wd3 = wt_d[nl, q].rearrange("p (c o) -> p c o", c=DC)
                nc.sync.dma_start(wts[nl][:, :, q * 512 : q * 512 + 512], wd3)

            for nl in range(NL):
                wt_chunk_dma(nl, 0)

            # PE p-state warmup burst during the startup DMA window, plus a
            # filler bank used to pace the PE so it never idles mid-kernel
            # (an idle gap resets the clock ramp and the whole queued burst
            # of real matmuls gets costed at the low-p-state rate).
            scr = const_pool.tile([128, 128], BF16, name="scr")
            nc.vector.memset(scr[:], 0.0)
            wps = psum_pool.tile([128, 1024], F32, name="ps", tag="ps")
            for _ in range(56):
                nc.tensor.matmul(
                    wps[0:64, 0:64], scr[:, 0:64], scr[:, 64:128], start=True, stop=True
                )

            def add_op(path, dst, src):
                if path == "V":
                    nc.vector.tensor_add(dst, dst, src)
                elif path == "M":
                    nc.gpsimd.dma_start(dst, src, accum_op=ADD)
                else:
                    nc.gpsimd.tensor_add(dst, dst, src)

            def body(iv=None):
                # ots with L2/L3 adds still pending, lagged TWO ots so that
                # accum-DMA leaf latency (~5-6us) never blocks the DVE queue
                pend = []

                def l2a(p):
                    add_op(_slot_pat(p[0], 4), p[2][1][:], p[2][3][:])

                def l2b(p):
                    add_op(_slot_pat(p[0], 5), p[1][:], p[2][5][:])

                def l3_out(p):
                    add_op(_slot_pat(p[0], 6), p[1][:], p[2][1][:])
                    nc.sync.dma_start(out_d[:, p[0] * B : p[0] * B + B], p[1][:])

                for ot in range(OT):
                    acc_t = None
                    ts = [None] * NL
                    for nl in range(NL):
                        ps = psum_pool.tile([128, 1024], F32, name="ps", tag="ps")
                        wt = wts[nl]
                        for ci in range(2):
                            for bt in range(2):
                                nc.tensor.matmul(
                                    ps[:, bt * 512 : bt * 512 + 512],
                                    wt[:, 2 * ci : 2 * ci + 2, ot * 128 : ot * 128 + 128],
                                    st[:, 2 * ci : 2 * ci + 2, bt * 512 : bt * 512 + 512],
                                    start=(ci == 0),
                                    stop=(ci == 1),
                                    perf_mode=DR,
                                )
                        b_ap = cb[:, nl * OT + ot : nl * OT + ot + 1]
                        if nl == 0:
                            acc_t = acc_pool.tile([128, B], BF16, name="acc", tag="acc")
                            d_ap = acc_t[:]
                        else:
                            ts[nl] = tmp_pool.tile(
                                [128, B], BF16, name="tmp", tag=f"tmp{nl}"
                            )
                            d_ap = ts[nl][:]
                        if _RELU_SCHED[ot * NL + nl] == "A":
                            nc.scalar.activation(d_ap, ps[:], RELU, bias=b_ap, scale=1.0)
                        else:
                            nc.vector.tensor_scalar(
                                d_ap, ps[:], b_ap, 0.0, op0=ADD, op1=MAX
                            )
                        # this ot's L1 adds + the 2-ot-lagged L2/L3 of pend[0]
                        old = pend[0] if len(pend) >= 2 else None
                        if nl == 1 and old is not None:
                            l2a(old)
                        elif nl == 3:
                            add_op(_slot_pat(ot, 0), ts[1][:], ts[2][:])     # L1a
                        elif nl == 4 and old is not None:
                            l2b(old)
                        elif nl == 5:
                            add_op(_slot_pat(ot, 1), ts[3][:], ts[4][:])     # L1b
                        elif nl == 6 and old is not None:
                            l3_out(old)
                            pend.pop(0)
                        elif nl == 7:
                            add_op(_slot_pat(ot, 2), ts[5][:], ts[6][:])     # L1c
                        # next-quad weight prefetch: 2 chunks per ot, so all
                        # 8 (branch, q) chunks land during quad q-1
                        q = ot // 4 + 1
                        if q < NQ and nl in (1, 5):
                            wt_chunk_dma((ot % 4) * 2 + (nl == 5), q)
                    add_op(_slot_pat(ot, 3), acc_t[:], ts[7][:])             # L1d
                    pend.append((ot, acc_t, ts))
                for p in pend:
                    l2a(p)
                    l2b(p)
                    l3_out(p)

            if repeat == 1:
                body()
            else:
                with tc.For_i(0, repeat, 1):
                    body()

    nc.compile()
    _cache[repeat] = nc
    return nc


def prep_inputs(semantic_vec, vertices, W, b):
    """Host-side layout transforms + fp8 quantization -> per-core inputs."""
    s64 = np.asarray(semantic_vec, dtype=np.float64)
    v64 = np.asarray(vertices, dtype=np.float64)
    W64 = np.asarray(W, dtype=np.float64)
    b64 = np.asarray(b, dtype=np.float64)

    # c[n, o] = b[n, o] - v[n] @ W[n].T  (exact, f64)
    c = b64 - np.einsum("nd,nod->no", v64, W64)

    # st8[p, c*B + bb] = fp8(s[bb, c*128+p])
    st8 = np.ascontiguousarray(
        s64.reshape(B, DC, 128).transpose(2, 1, 0).reshape(128, DC * B)
    ).astype(ml_dtypes.float8_e4m3fn)
    # wt8[n, q, p, c*512 + oo] = fp8(AW * W[n, q*512 + oo, c*128+p])
    # (o-quad-chunked so each (branch, quad) DMA is one contiguous run)
    wt8 = np.ascontiguousarray(
        (AW * W64)
        .reshape(N, NQ, 512, DC, 128)      # [n, q, oo, c, p]
        .transpose(0, 1, 4, 3, 2)          # [n, q, p, c, oo]
        .reshape(N, NQ, 128, DC * 512)
    ).astype(ml_dtypes.float8_e4m3fn)
    # cb[core, p, nl*OT + ot] = f32(AW * c[n0+nl, ot*128+p])
    cb = np.ascontiguousarray(
        (AW * c).reshape(N_CORES, NL, OT, 128).transpose(0, 3, 1, 2).reshape(N_CORES, 128, NL * OT)
    ).astype(np.float32)

    in_maps = []
    for core in range(N_CORES):
        in_maps.append(
            {
                "wt": wt8[core * NL : (core + 1) * NL],
                "st": st8,
                "cb": cb[core],
            }
        )
    return in_maps


def kernel(semantic_vec, vertices, W, b):
    nc = build(repeat=1)
    in_maps = prep_inputs(semantic_vec, vertices, W, b)
    res = run_bass_kernel_spmd(nc, in_maps, core_ids=list(range(N_CORES)))
    total = np.zeros((OUT, B), dtype=np.float32)
    for core in range(N_CORES):
        o = np.asarray(res.results[core]["out"]).astype(np.float32)
        # o[p, ot*B + bb] -> out[ot*128 + p, bb]
        total += o.reshape(128, OT, B).transpose(1, 0, 2).reshape(OUT, B)
    total *= np.float32(1.0 / AW)
    return np.ascontiguousarray(total.T)


# revision 42
# speedup vs baseline: 2.2297x; 1.1213x over previous
"""Trainium2 Bass kernel for the dense branch-MLP problem (fp8 DoubleRow).

Computes: out[b,o] = sum_n relu((s[b,:] - v[n,:]) @ W[n].T + bias[n])[o]
with B=1024, N=64, D=512, OUT=2048 in fp32, graded at rel_absmax < 2e-2.

Sharding: expert-style across the N=64 branch axis -> 8 branches per core.
Each core computes a full [B, OUT] partial sum over its 8 branches; the
host sums the 8 partials and descales (the unshard step).

Math restructure (host side):
  (s - v_n) @ W_n^T + b_n  ==  s @ W_n^T + c_n,   c_n = b_n - v_n @ W_n^T
so the device never materializes offsets; c_n folds into the relu bias.
Both s and (aw*W) are quantized to fp8 e4m3 on the host (aw=16 keeps W out
of the subnormal range); full-chain emulated rel_absmax = 1.35e-2.

Per-core device schedule (o-tile OUTER, branch inner, so each o-tile's
output DMA fires early and the serial DMA pipeline never backlogs at the
end):
  - per (ot, nl): 4 fp8 DoubleRow matmuls (0.5 cyc/row, 2 k-tiles each)
    accumulate psum [128,1024] = the branch response for one o-tile,
  - relu+bias psum -> bf16 on ACT (activation) or DVE (tensor_scalar
    (x+c) max 0), statically split,
  - the 7 branch-sum adds per o-tile run on three tracks: DVE
    tensor_tensor (bf16 2x mode), gpsimd accumulating DMAs, and gpsimd
    tensor_tensor; out DMA per o-tile right after its last add.
Weights stream as (branch, o-quad) chunks relaid out on the host so each
chunk is a contiguous 2KB-per-partition DMA.
"""

import numpy as np
import ml_dtypes

import concourse.bacc as bacc
import concourse.mybir as mybir
import concourse.tile as tile
from concourse.bass_utils import run_bass_kernel_spmd

B, N, D, OUT = 1024, 64, 512, 2048
N_CORES = 8
NL = N // N_CORES  # branches per core (8)
DC = D // 128      # contraction k-tiles (4)
OT = OUT // 128    # o tiles (16)
NQ = 4             # weight-stream quads (OT/4)
AW = 16.0          # host-side weight scale before fp8 quantization

F32 = mybir.dt.float32
BF16 = mybir.dt.bfloat16
F8 = mybir.dt.float8e4
RELU = mybir.ActivationFunctionType.Relu
DR = mybir.MatmulPerfMode.DoubleRow
ADD = mybir.AluOpType.add
MAX = mybir.AluOpType.max


def _spread(total, picks):
    """Boolean mask of length `total` with `picks` Trues spread evenly."""
    return [(i * picks) // total != ((i - 1) * picks) // total for i in range(total)]


# -- static schedules (tuned against the cost-model sim) --------------------
# relu engine per unit (unit = ot*8+nl): 'A' = ACT activation, 'D' = DVE.
N_DVE_RELU = 47
_RELU_SCHED = ["D" if d else "A" for d in _spread(128, N_DVE_RELU)]
_RELU_SCHED[0], _RELU_SCHED[1] = "A", "D"  # unit 0 on ACT (DVE starts busy)

# Branch-sum per ot is a depth-3 tree over {acc(=relu nl0), t1..t7}:
#   L1: t1+=t2   t3+=t4   t5+=t6   acc+=t7     (emitted as relus complete)
#   L2: t1+=t3   acc+=t5
#   L3: acc+=t1  -> out DMA
# Each slot's engine per ot: 'V' = DVE tensor_tensor, 'M' = gpsimd
# accumulating DMA (high latency, leaves only), 'P' = gpsimd TT.
def _slot_pat(ot, slot):
    if ot >= OT - 2:
        # keep the serial DMA pipeline + pool clear for the final out DMAs
        return "V"
    if ot <= 2:
        # weight prefetch owns the DMA pipeline early: no accum-DMAs yet,
        # lean on the (still idle) pool and DVE instead
        return ("P", "P", "P", "V", "V", "V", "V")[slot]
    if slot in (0, 1):           # L1a, L1b (leaves: latency-tolerant)
        return "M"
    if slot == 2:                # L1c
        return "P" if ot in (5, 10) else "M"
    if slot == 3:                # L1d (acc chain start, emitted at ot end)
        return "V"
    if slot == 4:                # L2a (lagged two ots)
        return "M" if ot % 2 else "V"
    if slot == 5:                # L2b (lagged two ots)
        return "V" if ot % 2 else "M"
    return "V"                   # L3 (feeds the out DMA)

_cache = {}


def build(repeat: int = 1):
    """Build + compile the per-core Bass program. Cached per `repeat`."""
    if repeat in _cache:
        return _cache[repeat]

    nc = bacc.Bacc(
        "TRN2",
        target_bir_lowering=False,
        debug=False,
        num_devices=N_CORES,
    )

    # weights pre-chunked on host: [nl, quad, 128, DC*512] fp8
    wt_d = nc.dram_tensor("wt", [NL, NQ, 128, DC * 512], F8, kind="ExternalInput").ap()
    st_d = nc.dram_tensor("st", [128, DC * B], F8, kind="ExternalInput").ap()
    cb_d = nc.dram_tensor("cb", [128, NL * OT], F32, kind="ExternalInput").ap()
    out_d = nc.dram_tensor("out", [128, OT * B], BF16, kind="ExternalOutput").ap()

    with tile.TileContext(nc) as tc:
        with (
            tc.tile_pool(name="const", bufs=1) as const_pool,
            tc.tile_pool(name="acc", bufs=4) as acc_pool,
            tc.tile_pool(name="tmp", bufs=4) as tmp_pool,
            tc.tile_pool(name="wt", bufs=1) as wt_pool,
            tc.tile_pool(name="psum", bufs=4, space="PSUM") as psum_pool,
        ):
            # Startup order: cb gates the first relu drain; the first two st
            # k-tile pairs + branch-0's quad-0 chunk gate the first matmuls.
            cb = const_pool.tile([128, NL * OT], F32, name="cb")
            nc.sync.dma_start(cb[:], cb_d[:])
            st = const_pool.tile([128, DC, B], F8, name="st")
            st_d3 = st_d.rearrange("p (c b) -> p c b", c=DC)

            # one resident weight tile per branch, filled quad-by-quad
            wts = [
                wt_pool.tile([128, DC, OUT], F8, name=f"wt{nl}", tag=f"wt{nl}")
                for nl in range(NL)
            ]

            def wt_chunk_dma(nl, q):
                wd3 = wt_d[nl, q].rearrange("p (c o) -> p c o", c=DC)
                nc.sync.dma_start(wts[nl][:, :, q * 512 : q * 512 + 512], wd3)

            nc.sync.dma_start(st[:, 0:2, :], st_d3[:, 0:2, :])
            wt_chunk_dma(0, 0)
            nc.sync.dma_start(st[:, 2:4, :], st_d3[:, 2:4, :])
            for nl in range(1, NL):
                wt_chunk_dma(nl, 0)

            # PE p-state warmup burst during the startup DMA window, plus a
            # filler bank used to pace the PE so it never idles mid-kernel
            # (an idle gap resets the clock ramp and the whole queued burst
            # of real matmuls gets costed at the low-p-state rate).
            scr = const_pool.tile([128, 128], BF16, name="scr")
            nc.vector.memset(scr[:], 0.0)
            wps = psum_pool.tile([128, 1024], F32, name="ps", tag="ps")
            for _ in range(56):
                nc.tensor.matmul(
                    wps[0:64, 0:64], scr[:, 0:64], scr[:, 64:128], start=True, stop=True
                )

            def add_op(path, dst, src):
                if path == "V":
                    nc.vector.tensor_add(dst, dst, src)
                elif path == "M":
                    nc.gpsimd.dma_start(dst, src, accum_op=ADD)
                else:
                    nc.gpsimd.tensor_add(dst, dst, src)

            def body(iv=None):
                # ots with L2/L3 adds still pending, lagged TWO ots so that
                # accum-DMA leaf latency (~5-6us) never blocks the DVE queue
                pend = []

                def l2a(p):
                    add_op(_slot_pat(p[0], 4), p[2][1][:], p[2][3][:])

                def l2b(p):
                    add_op(_slot_pat(p[0], 5), p[1][:], p[2][5][:])

                def l3_out(p):
                    add_op(_slot_pat(p[0], 6), p[1][:], p[2][1][:])
                    nc.sync.dma_start(out_d[:, p[0] * B : p[0] * B + B], p[1][:])

                for ot in range(OT):
                    acc_t = None
                    ts = [None] * NL
                    for nl in range(NL):
                        ps = psum_pool.tile([128, 1024], F32, name="ps", tag="ps")
                        wt = wts[nl]
                        for ci in range(2):
                            for bt in range(2):
                                nc.tensor.matmul(
                                    ps[:, bt * 512 : bt * 512 + 512],
                                    wt[:, 2 * ci : 2 * ci + 2, ot * 128 : ot * 128 + 128],
                                    st[:, 2 * ci : 2 * ci + 2, bt * 512 : bt * 512 + 512],
                                    start=(ci == 0),
                                    stop=(ci == 1),
                                    perf_mode=DR,
                                )
                        b_ap = cb[:, nl * OT + ot : nl * OT + ot + 1]
                        if nl == 0:
                            acc_t = acc_pool.tile([128, B], BF16, name="acc", tag="acc")
                            d_ap = acc_t[:]
                        else:
                            ts[nl] = tmp_pool.tile(
                                [128, B], BF16, name="tmp", tag=f"tmp{nl}"
                            )
                            d_ap = ts[nl][:]
                        if _RELU_SCHED[ot * NL + nl] == "A":
                            nc.scalar.activation(d_ap, ps[:], RELU, bias=b_ap, scale=1.0)
                        else:
                            nc.vector.tensor_scalar(
                                d_ap, ps[:], b_ap, 0.0, op0=ADD, op1=MAX
                            )
                        # this ot's L1 adds + the 2-ot-lagged L2/L3 of pend[0]
                        # (the final ot drains BOTH pending trees)
                        last = ot == OT - 1
                        old = pend[0] if len(pend) >= 2 else None
                        if nl == 1 and old is not None:
                            l2a(old)
                        elif nl == 2 and last and old is not None:
                            l2b(old)
                        elif nl == 3:
                            add_op(_slot_pat(ot, 0), ts[1][:], ts[2][:])     # L1a
                            if last and old is not None:
                                l3_out(old)
                                pend.pop(0)
                                old = pend[0]
                        elif nl == 4 and old is not None:
                            l2b(old) if not last else l2a(old)
                        elif nl == 5:
                            add_op(_slot_pat(ot, 1), ts[3][:], ts[4][:])     # L1b
                            if last and old is not None:
                                l2b(old)
                        elif nl == 6 and old is not None:
                            l3_out(old)
                            pend.pop(0)
                        elif nl == 7:
                            add_op(_slot_pat(ot, 2), ts[5][:], ts[6][:])     # L1c
                        # next-quad weight prefetch: 2 chunks per ot, so all
                        # 8 (branch, q) chunks land during quad q-1
                        q = ot // 4 + 1
                        if q < NQ and nl in (1, 5):
                            wt_chunk_dma((ot % 4) * 2 + (nl == 5), q)
                    add_op(_slot_pat(ot, 3), acc_t[:], ts[7][:])             # L1d
                    pend.append((ot, acc_t, ts))
                for p in pend:
                    l2a(p)
                    l2b(p)
                    l3_out(p)

            if repeat == 1:
                body()
            else:
                with tc.For_i(0, repeat, 1):
                    body()

    nc.compile()
    _cache[repeat] = nc
    return nc


def prep_inputs(semantic_vec, vertices, W, b):
    """Host-side layout transforms + fp8 quantization -> per-core inputs."""
    s64 = np.asarray(semantic_vec, dtype=np.float64)
    v64 = np.asarray(vertices, dtype=np.float64)
    W64 = np.asarray(W, dtype=np.float64)
    b64 = np.asarray(b, dtype=np.float64)

    # c[n, o] = b[n, o] - v[n] @ W[n].T  (exact, f64)
    c = b64 - np.einsum("nd,nod->no", v64, W64)

    # st8[p, c*B + bb] = fp8(s[bb, c*128+p])
    st8 = np.ascontiguousarray(
        s64.reshape(B, DC, 128).transpose(2, 1, 0).reshape(128, DC * B)
    ).astype(ml_dtypes.float8_e4m3fn)
    # wt8[n, q, p, c*512 + oo] = fp8(AW * W[n, q*512 + oo, c*128+p])
    # (o-quad-chunked so each (branch, quad) DMA is one contiguous run)
    wt8 = np.ascontiguousarray(
        (AW * W64)
        .reshape(N, NQ, 512, DC, 128)      # [n, q, oo, c, p]
        .transpose(0, 1, 4, 3, 2)          # [n, q, p, c, oo]
        .reshape(N, NQ, 128, DC * 512)
    ).astype(ml_dtypes.float8_e4m3fn)
    # cb[core, p, nl*OT + ot] = f32(AW * c[n0+nl, ot*128+p])
    cb = np.ascontiguousarray(
        (AW * c).reshape(N_CORES, NL, OT, 128).transpose(0, 3, 1, 2).reshape(N_CORES, 128, NL * OT)
    ).astype(np.float32)

    in_maps = []
    for core in range(N_CORES):
        in_maps.append(
            {
                "wt": wt8[core * NL : (core + 1) * NL],
                "st": st8,
                "cb": cb[core],
            }
        )
    return in_maps


def kernel(semantic_vec, vertices, W, b):
    nc = build(repeat=1)
    in_maps = prep_inputs(semantic_vec, vertices, W, b)
    res = run_bass_kernel_spmd(nc, in_maps, core_ids=list(range(N_CORES)))
    total = np.zeros((OUT, B), dtype=np.float32)
    for core in range(N_CORES):
        o = np.asarray(res.results[core]["out"]).astype(np.float32)
        # o[p, ot*B + bb] -> out[ot*128 + p, bb]
        total += o.reshape(128, OT, B).transpose(1, 0, 2).reshape(OUT, B)
    total *= np.float32(1.0 / AW)
    return np.ascontiguousarray(total.T)


# revision 53
# speedup vs baseline: 2.3893x; 1.0716x over previous
"""Trainium2 Bass kernel for the dense branch-MLP problem (fp8 DoubleRow).

Computes: out[b,o] = sum_n relu((s[b,:] - v[n,:]) @ W[n].T + bias[n])[o]
with B=1024, N=64, D=512, OUT=2048 in fp32, graded at rel_absmax < 2e-2.

Sharding: expert-style across the N=64 branch axis -> 8 branches per core.
Each core computes a full [B, OUT] partial sum over its 8 branches; the
host sums the 8 partials and descales (the unshard step).

Math restructure (host side):
  (s - v_n) @ W_n^T + b_n  ==  s @ W_n^T + c_n,   c_n = b_n - v_n @ W_n^T
Both s and (aw*W) are quantized to fp8 e4m3 on the host (aw=16 keeps W out
of the subnormal range); full-chain emulated rel_absmax ~= 1.35e-2.

FLIPPED-LAYOUT schedule: psum partitions = batch, free = output. Each unit
(nl, bt, h) = one branch x one 128-batch tile x one 1024-wide output half
in a [128, 1024] psum span (2 banks, 4-deep ring):
  - per 512-wide psum bank: one partition-1 fp8 DoubleRow "bias" matmul
    (ones stationary, c_n hi/lo fp8 rows moving, operands parked at
    partition 32*oc) pre-loads c_n into psum, then 2 DoubleRow matmuls
    accumulate s @ W_n^T on top,
  - the drain is then BIAS-FREE, so DVE fuses relu+accumulate in one
    scalar_tensor_tensor pass (acc[bt] = (psum MAX 0) ADD acc[bt]) and ACT
    does plain relus into tmp, added into acc[bt] via DVE tensor_tensor,
    gpsimd accumulating DMAs, or gpsimd tensor_tensor,
  - each acc[bt] half-chain has one link per branch, ~11us apart: add
    latency is invisible. Out DMA per bt right after branch 7's links.
PE is the bottleneck (512 branch + 256 bias DoubleRow matmuls, ~89us) and
runs near-continuously, keeping the clock-ramp p-state at full speed.
"""

import numpy as np
import ml_dtypes

import concourse.bacc as bacc
import concourse.mybir as mybir
import concourse.tile as tile
from concourse.bass_utils import run_bass_kernel_spmd

B, N, D, OUT = 1024, 64, 512, 2048
N_CORES = 8
NL = N // N_CORES  # branches per core (8)
DC = D // 128      # contraction k-tiles (4)
BT = B // 128      # batch tiles (8)
NQ = 4             # weight-stream quads
AW = 16.0          # host-side weight scale before fp8 quantization

F32 = mybir.dt.float32
BF16 = mybir.dt.bfloat16
F8 = mybir.dt.float8e4
RELU = mybir.ActivationFunctionType.Relu
DR = mybir.MatmulPerfMode.DoubleRow
ADD = mybir.AluOpType.add
MAX = mybir.AluOpType.max

# -- static schedules -------------------------------------------------------
# Unit (nl, bt, h) drain: 'D' = DVE fused stt, 'A' = ACT relu (+add, nl>0).
N_D_UNITS = 54


def _spread(total, picks):
    return [(i * picks) // total != ((i - 1) * picks) // total for i in range(total)]


_DMASK = _spread(128, N_D_UNITS)
_FORM = {}
for _nl in range(NL):
    for _bt in range(BT):
        for _h in range(2):
            _FORM[(_nl, _bt, _h)] = "D" if _DMASK[_nl * 16 + _bt * 2 + _h] else "A"
_FORM[(7, 6, 1)] = _FORM[(7, 7, 0)] = _FORM[(7, 7, 1)] = "A"


def _a_add_path(nl, bt, k):
    """Path for an A-unit's add into acc[bt] ('V'/'M'/'P'). k spreads it."""
    if nl <= 1:
        return "P" if bt % 2 else "V"
    if nl >= NL - 1:
        return "V"
    return "M"


_cache = {}


def build(repeat: int = 1):
    """Build + compile the per-core Bass program. Cached per `repeat`."""
    if repeat in _cache:
        return _cache[repeat]

    nc = bacc.Bacc(
        "TRN2",
        target_bir_lowering=False,
        debug=False,
        num_devices=N_CORES,
    )

    # weights pre-chunked on host: [nl, quad, 128, DC*512] fp8
    wt_d = nc.dram_tensor("wt", [NL, NQ, 128, DC * 512], F8, kind="ExternalInput").ap()
    st_d = nc.dram_tensor("st", [128, DC * B], F8, kind="ExternalInput").ap()
    # bias rows: partition p(oc) in {0,32,64}, slot s(oc): [nl, hi/lo, 512]
    c8_d = nc.dram_tensor("c8", [128, NL * 2 * 2 * 512], F8, kind="ExternalInput").ap()
    ones_d = nc.dram_tensor("ones", [128, 256], F8, kind="ExternalInput").ap()
    out_d = nc.dram_tensor("out", [128, BT * OUT], BF16, kind="ExternalOutput").ap()

    with tile.TileContext(nc) as tc:
        with (
            tc.tile_pool(name="const", bufs=1) as const_pool,
            tc.tile_pool(name="acc", bufs=1) as acc_pool,
            tc.tile_pool(name="tmp", bufs=4) as tmp_pool,
            tc.tile_pool(name="wt", bufs=1) as wt_pool,
            tc.tile_pool(name="psum", bufs=4, space="PSUM") as psum_pool,
        ):
            ones = const_pool.tile([128, 256], F8, name="ones")
            nc.sync.dma_start(ones[:], ones_d[:])
            c8 = const_pool.tile([128, NL, 2, 2, 512], F8, name="c8")
            c8_d5 = c8_d.rearrange("p (n k s o) -> p n k s o", n=NL, k=2, s=2)

            def c8_chunk_dma(nl):
                nc.sync.dma_start(c8[:, nl], c8_d5[:, nl])

            st = const_pool.tile([128, DC, B], F8, name="st")
            st_d3 = st_d.rearrange("p (c b) -> p c b", c=DC)

            # one resident weight tile per branch, filled quad-by-quad
            wts = [
                wt_pool.tile([128, DC, OUT], F8, name=f"wt{nl}", tag=f"wt{nl}")
                for nl in range(NL)
            ]

            def wt_chunk_dma(nl, q):
                wd3 = wt_d[nl, q].rearrange("p (c o) -> p c o", c=DC)
                nc.sync.dma_start(wts[nl][:, :, q * 512 : q * 512 + 512], wd3)

            c8_chunk_dma(0)
            for c in range(DC):
                nc.sync.dma_start(st[:, c], st_d3[:, c])
            wt_chunk_dma(0, 0)
            wt_chunk_dma(0, 1)
            c8_chunk_dma(1)
            wt_chunk_dma(0, 2)
            wt_chunk_dma(0, 3)

            # PE p-state warmup burst during the startup DMA window.
            scr = const_pool.tile([128, 128], BF16, name="scr")
            nc.vector.memset(scr[:], 0.0)
            wps = psum_pool.tile([128, 1024], F32, name="ps", tag="ps")
            for _ in range(56):
                nc.tensor.matmul(
                    wps[0:64, 0:64], scr[:, 0:64], scr[:, 64:128], start=True, stop=True
                )

            # per-bt bf16 accumulators, resident across all branches
            accs = [
                acc_pool.tile([128, OUT], BF16, name=f"acc{bt}", tag=f"acc{bt}")
                for bt in range(BT)
            ]

            def add_op(path, dst, src):
                if path == "V":
                    nc.vector.tensor_add(dst, dst, src)
                elif path == "M":
                    nc.gpsimd.dma_start(dst, src, accum_op=ADD)
                else:
                    nc.gpsimd.tensor_add(dst, dst, src)

            def body(iv=None):
                a_pend = []  # (nl, bt, h, tmp, k) adds awaiting emission
                a_cnt = 0

                def flush_one():
                    anl, abt, ah, at, k = a_pend.pop(0)
                    dst = accs[abt][:, ah * 1024 : ah * 1024 + 1024]
                    add_op(_a_add_path(anl, abt, k), dst, at[:])

                for nl in range(NL):
                    for bt in range(BT):
                        for h in range(2):
                            ps = psum_pool.tile([128, 1024], F32, name="ps", tag="ps")
                            wt = wts[nl]
                            for j in range(2):
                                oc = 2 * h + j
                                osl = slice(oc * 512, oc * 512 + 512)
                                psl = slice(j * 512, j * 512 + 512)
                                p0, slot = ((0, 0), (32, 0), (64, 0), (0, 1))[oc]
                                ones3 = ones[p0 : p0 + 1, :].rearrange(
                                    "p (k f) -> p k f", k=2
                                )
                                nc.tensor.matmul(
                                    ps[:, psl],
                                    ones3,
                                    c8[p0 : p0 + 1, nl, :, slot, :],
                                    start=True,
                                    stop=False,
                                    perf_mode=DR,
                                )
                                for ci in range(2):
                                    nc.tensor.matmul(
                                        ps[:, psl],
                                        st[:, 2 * ci : 2 * ci + 2, bt * 128 : bt * 128 + 128],
                                        wt[:, 2 * ci : 2 * ci + 2, osl],
                                        start=False,
                                        stop=(ci == 1),
                                        perf_mode=DR,
                                    )
                            acc_h = accs[bt][:, h * 1024 : h * 1024 + 1024]
                            if _FORM[(nl, bt, h)] == "D":
                                if nl == 0:
                                    nc.vector.tensor_scalar(
                                        acc_h, ps[:], 0.0, None, op0=MAX
                                    )
                                else:
                                    nc.vector.scalar_tensor_tensor(
                                        acc_h, ps[:], 0.0, acc_h, op0=MAX, op1=ADD
                                    )
                            else:
                                if nl == 0:
                                    nc.scalar.activation(
                                        acc_h, ps[:], RELU, bias=0.0, scale=1.0
                                    )
                                else:
                                    t = tmp_pool.tile(
                                        [128, 1024], BF16, name="tmp", tag=f"tmp{(bt * 2 + h) % 4}"
                                    )
                                    nc.scalar.activation(
                                        t[:], ps[:], RELU, bias=0.0, scale=1.0
                                    )
                                    a_pend.append((nl, bt, h, t, a_cnt))
                                    a_cnt += 1
                            # lagged A-adds (~2 units behind)
                            if len(a_pend) >= 3:
                                flush_one()
                            # out DMA per half as branch 7's link completes
                            if nl == NL - 1:
                                for item in [x for x in a_pend if x[1] == bt and x[2] == h]:
                                    a_pend.remove(item)
                                    dst = accs[bt][:, item[2] * 1024 : item[2] * 1024 + 1024]
                                    add_op(_a_add_path(item[0], bt, item[4]), dst, item[3][:])
                                nc.sync.dma_start(
                                    out_d[:, bt * OUT + h * 1024 : bt * OUT + h * 1024 + 1024],
                                    accs[bt][:, h * 1024 : h * 1024 + 1024],
                                )
                        # weight + bias prefetch for the next branch
                        if nl < NL - 1 and bt in (1, 3, 5, 7):
                            wt_chunk_dma(nl + 1, (bt - 1) // 2)
                            if bt == 1 and nl < NL - 2:
                                c8_chunk_dma(nl + 2)
                for item in a_pend:
                    dst = accs[item[1]][:, item[2] * 1024 : item[2] * 1024 + 1024]
                    add_op(_a_add_path(item[0], item[1], item[4]), dst, item[3][:])

            if repeat == 1:
                body()
            else:
                with tc.For_i(0, repeat, 1):
                    body()

    nc.compile()
    _cache[repeat] = nc
    return nc


def prep_inputs(semantic_vec, vertices, W, b):
    """Host-side layout transforms + fp8 quantization -> per-core inputs."""
    s64 = np.asarray(semantic_vec, dtype=np.float64)
    v64 = np.asarray(vertices, dtype=np.float64)
    W64 = np.asarray(W, dtype=np.float64)
    b64 = np.asarray(b, dtype=np.float64)
    f8 = ml_dtypes.float8_e4m3fn

    # c[n, o] = b[n, o] - v[n] @ W[n].T  (exact, f64), AW-scaled, fp8 hi/lo
    c = AW * (b64 - np.einsum("nd,nod->no", v64, W64))
    chi = c.astype(f8)
    clo = (c - chi.astype(np.float64)).astype(f8)
    # c8[core][p(oc), ((nl*2 + k)*2 + s(oc))*512 + j] = (hi,lo)[k][n, oc*512+j]
    c8 = np.zeros((N_CORES, 128, NL * 2 * 2 * 512), dtype=f8)
    _OCMAP = ((0, 0), (32, 0), (64, 0), (0, 1))
    for nl in range(NL):
        for k, arr in enumerate((chi, clo)):
            a4 = arr.reshape(N_CORES, NL, 4, 512)
            for oc in range(4):
                p0, s = _OCMAP[oc]
                base = ((nl * 2 + k) * 2 + s) * 512
                c8[:, p0, base : base + 512] = a4[:, nl, oc, :]

    # st8[p, c*B + bb] = fp8(s[bb, c*128+p])
    st8 = np.ascontiguousarray(
        s64.reshape(B, DC, 128).transpose(2, 1, 0).reshape(128, DC * B)
    ).astype(f8)
    # wt8[n, q, p, c*512 + oo] = fp8(AW * W[n, q*512 + oo, c*128+p])
    wt8 = np.ascontiguousarray(
        (AW * W64)
        .reshape(N, NQ, 512, DC, 128)      # [n, q, oo, c, p]
        .transpose(0, 1, 4, 3, 2)          # [n, q, p, c, oo]
        .reshape(N, NQ, 128, DC * 512)
    ).astype(f8)
    ones = np.ones((128, 256), dtype=f8)

    in_maps = []
    for core in range(N_CORES):
        in_maps.append(
            {
                "wt": wt8[core * NL : (core + 1) * NL],
                "st": st8,
                "c8": c8[core],
                "ones": ones,
            }
        )
    return in_maps


def kernel(semantic_vec, vertices, W, b):
    nc = build(repeat=1)
    in_maps = prep_inputs(semantic_vec, vertices, W, b)
    res = run_bass_kernel_spmd(nc, in_maps, core_ids=list(range(N_CORES)))
    total = np.zeros((B, OUT), dtype=np.float32)
    for core in range(N_CORES):
        o = np.asarray(res.results[core]["out"]).astype(np.float32)
        # o[p, bt*OUT + oo] -> out[bt*128 + p, oo]
        total += o.reshape(128, BT, OUT).transpose(1, 0, 2).reshape(B, OUT)
    total *= np.float32(1.0 / AW)
    return np.ascontiguousarray(total)
